# revision 1
# baseline (speedup 1.0000x reference)
"""2-layer GCN (PyG GCNConv semantics) on 8 Trainium2 NeuronCores.

Distribution: destination-node sharding (12500 nodes/core), edges
partitioned by dst and sorted; params replicated; layer-2 input (p = h@W2)
exchanged via AllGather of per-core shards.

Per-core pipeline (all fp32):
  - Edges of each 128-dst block are grouped by source table chunk
    (int16 gather indices address <=32768 rows) and packed into 128-slot
    bins.  dma_gather (<=1024 idxs/call) pulls source rows into SBUF,
    one row per (partition, free-slot).
  - Per bin, a scatter matrix S[slot, dst_local] = norm_e (one-hot row
    per slot) is built with a single DVE tensor_scalar
    (iota == dst_local) * norm, and the PE accumulates
    agg^T[feat, dst] += msgs^T @ S in PSUM across the block's bins.
  - z = W1^T agg (PE), h = relu(z + b1) (ScalarE, bias per-partition),
    p^T = W2^T h (PE), PE-transpose back to row layout, DMA to an
    internal DRAM shard, AllGather -> p_full.
  - Layer 2 repeats the gather/scatter over p_full, adds b2, transposes,
    writes fp32 output rows.

kernel(**inputs) takes FULL inputs, returns the FULL [N, 64] f32 output.
Set GCN_TRACE=1 to capture an NTFF profile (exec time in LAST_EXEC_NS).
"""

import math
import os
import sys
import types

import numpy as np

P = 128
NCORES = 8
CHUNK = 32768          # int16 index range per gather table chunk
MAX_IDXS = 1024        # dma_gather per-call limit on this HW


# --------------------------------------------------------------------------
# host-side preprocessing
# --------------------------------------------------------------------------

def _plan_layer(src_tab, dst_loc_core, norm_core, blk_of_core, B, n_tab_rows):
    """Uniform call plan + per-core slot arrays for one layer's gathers."""
    nchunks = (n_tab_rows + CHUNK - 1) // CHUNK
    sizes = np.zeros((NCORES, B, nchunks), np.int64)
    groups = [[[None] * nchunks for _ in range(B)] for _ in range(NCORES)]
    for c in range(NCORES):
        st = src_tab[c]
        ch = st // CHUNK
        key = blk_of_core[c] * nchunks + ch
        order = np.argsort(key, kind="stable")
        key_s = key[order]
        bounds = np.searchsorted(key_s, np.arange(B * nchunks + 1))
        for b in range(B):
            for k in range(nchunks):
                s0, s1 = bounds[b * nchunks + k], bounds[b * nchunks + k + 1]
                if s1 > s0:
                    g = order[s0:s1]
                    g = g[np.argsort(st[g], kind="stable")]  # src-sorted
                    groups[c][b][k] = g
                    sizes[c, b, k] = s1 - s0

    cap = sizes.max(axis=0)                       # [B, nchunks]
    cap = ((cap + P - 1) // P) * P                # round to whole bins

    # call plan (block, chunk, cap_sub, idx_col0, bin0, slot_off); uniform
    plan = []
    icol = 0
    mbin = 0
    for b in range(B):
        for k in range(nchunks):
            rem = int(cap[b, k])
            off = 0
            while rem > 0:
                sub = min(rem, MAX_IDXS)
                plan.append((b, k, sub, icol, mbin, off))
                icol += sub // 16
                mbin += sub // P
                rem -= sub
                off += sub
    icols, nbins = icol, mbin

    idx16 = np.zeros((NCORES, P, max(icols, 1)), np.int16)
    meta = np.zeros((NCORES, P, 2 * max(nbins, 1)), np.float32)
    meta[:, :, 0::2] = -1.0  # dst_local pad -> S row all-zero
    for c in range(NCORES):
        for (b, k, sub, ic, mb, off) in plan:
            g = groups[c][b][k]
            if g is None:
                continue
            lo = off
            hi = min(off + sub, len(g))
            if hi <= lo:
                continue
            e = g[lo:hi]
            n = hi - lo
            st = (src_tab[c][e] - k * CHUNK).astype(np.int16)
            i = np.arange(n)
            idx16[c, i % 16, ic + i // 16] = st
            meta[c, i % P, 2 * (mb + i // P)] = dst_loc_core[c][e]
            meta[c, i % P, 2 * (mb + i // P) + 1] = norm_core[c][e]
    for gshift in range(1, 8):  # idxs replicated across 16-partition groups
        idx16[:, gshift * 16 : (gshift + 1) * 16, :] = idx16[:, :16, :]

    return dict(plan=plan, icols=icols, nbins=nbins, idx16=idx16, meta=meta,
                nchunks=nchunks)


def _preprocess(x, edge_index):
    N = x.shape[0]
    src = np.concatenate([np.asarray(edge_index[0]), np.arange(N)]).astype(np.int64)
    dst = np.concatenate([np.asarray(edge_index[1]), np.arange(N)]).astype(np.int64)
    deg = np.bincount(dst, minlength=N).astype(np.float32)
    dinv = np.where(deg > 0, 1.0 / np.sqrt(deg), 0.0).astype(np.float32)
    norm = (dinv[src] * dinv[dst]).astype(np.float32)

    assert N % NCORES == 0
    NPC = N // NCORES
    B = (NPC + P - 1) // P
    PADN = B * P

    core_of = dst // NPC
    srcs, dstls, norms, blks = [], [], [], []
    for c in range(NCORES):
        m = core_of == c
        srcs.append(src[m])
        dl = dst[m] - c * NPC
        blks.append(dl // P)
        dstls.append((dl % P).astype(np.float32))
        norms.append(norm[m])

    l1 = _plan_layer(srcs, dstls, norms, blks, B, N)
    srcp = [(s // NPC) * PADN + (s % NPC) for s in srcs]
    l2 = _plan_layer(srcp, dstls, norms, blks, B, NCORES * PADN)
    return dict(NPC=NPC, B=B, PADN=PADN, l1=l1, l2=l2)


# --------------------------------------------------------------------------
# bass program
# --------------------------------------------------------------------------

def _build(N, IN, HID, OUT, B, PADN, l1, l2):
    import concourse.bass as bass
    import concourse.bacc as bacc
    import concourse.mybir as mybir
    import concourse.tile as tile
    from concourse.masks import make_identity

    f32 = mybir.dt.float32
    i32 = mybir.dt.int32
    i16 = mybir.dt.int16
    eq = mybir.AluOpType.is_equal
    mul = mybir.AluOpType.mult
    add = mybir.AluOpType.add
    GT = NCORES * PADN
    ICOLS = max(l1["icols"], l2["icols"], 1)
    NBINS = max(l1["nbins"], l2["nbins"], 1)

    nc = bacc.Bacc("TRN2", num_devices=NCORES)
    xt = nc.dram_tensor("xt", [N, IN], f32, kind="ExternalInput")
    idx1 = nc.dram_tensor("idx1", [P, max(l1["icols"], 1)], i16, kind="ExternalInput")
    idx2 = nc.dram_tensor("idx2", [P, max(l2["icols"], 1)], i16, kind="ExternalInput")
    met1 = nc.dram_tensor("met1", [P, 2 * max(l1["nbins"], 1)], f32, kind="ExternalInput")
    met2 = nc.dram_tensor("met2", [P, 2 * max(l2["nbins"], 1)], f32, kind="ExternalInput")
    w1 = nc.dram_tensor("w1", [IN, HID], f32, kind="ExternalInput")
    w2 = nc.dram_tensor("w2", [HID, OUT], f32, kind="ExternalInput")
    b1t = nc.dram_tensor("b1t", [HID, 1], f32, kind="ExternalInput")
    b2t = nc.dram_tensor("b2t", [OUT, 1], f32, kind="ExternalInput")
    p_shard = nc.dram_tensor("p_shard", [PADN, OUT], f32, kind="Internal")
    p_full = nc.dram_tensor("p_full", [GT, OUT], f32, kind="Internal",
                            addr_space="Shared")
    outt = nc.dram_tensor("outt", [PADN, OUT], f32, kind="ExternalOutput")

    with tile.TileContext(nc) as tc:
        with (
            tc.tile_pool(name="const", bufs=1) as cpool,
            tc.tile_pool(name="meta", bufs=1) as mpool,
            tc.tile_pool(name="gath", bufs=4) as gpool,
            tc.tile_pool(name="work", bufs=4) as spool,
            tc.tile_pool(name="hacc", bufs=B) as hpool,
            tc.tile_pool(name="psA", bufs=2, space="PSUM") as psA,
            tc.tile_pool(name="psB", bufs=2, space="PSUM") as psB,
            tc.tile_pool(name="psC", bufs=2, space="PSUM") as psC,
        ):
            w1_sb = cpool.tile([IN, HID], f32)
            nc.sync.dma_start(w1_sb[:], w1[:])
            w2_sb = cpool.tile([HID, OUT], f32)
            nc.sync.dma_start(w2_sb[:], w2[:])
            b1_sb = cpool.tile([HID, 1], f32)
            nc.sync.dma_start(b1_sb[:], b1t[:])
            b2_sb = cpool.tile([OUT, 1], f32)
            nc.sync.dma_start(b2_sb[:], b2t[:])
            iota_i = cpool.tile([P, P], i32)
            nc.gpsimd.iota(iota_i[:], pattern=[[1, P]], base=0, channel_multiplier=0)
            iota_f = cpool.tile([P, P], f32)
            nc.vector.tensor_copy(iota_f[:], iota_i[:])
            idf = cpool.tile([P, P], f32)
            make_identity(nc, idf[:])

            idx_sb = mpool.tile([P, ICOLS], i16)
            meta_sb = mpool.tile([P, 2 * NBINS], f32)
            rows_st = mpool.tile([P, B * OUT], f32)

            h_tiles = []

            def layer(lp, table, elem, idx_dram, met_dram, out_part, epilogue):
                nc.sync.dma_start(idx_sb[:, : max(lp["icols"], 1)], idx_dram[:])
                nc.sync.dma_start(meta_sb[:, : 2 * max(lp["nbins"], 1)], met_dram[:])
                by_block = {}
                for call in lp["plan"]:
                    by_block.setdefault(call[0], []).append(call)
                for b in range(B):
                    calls = by_block.get(b, [])
                    nbins_b = sum(cc[2] // P for cc in calls)
                    if nbins_b == 0:
                        agg_sb = spool.tile([out_part, P], f32, tag="aggsb")
                        nc.vector.memset(agg_sb[:], 0.0)
                        epilogue(b, agg_sb, None)
                        continue
                    agg_ps = psA.tile([out_part, P], f32, tag="agg")
                    bin_i = 0
                    for (bb, k, sub, ic, mb, off) in calls:
                        lo = k * CHUNK
                        hi = min(lo + CHUNK, table.shape[0])
                        msgs = gpool.tile([P, (sub // P) * elem], f32, tag="msgs")
                        nc.gpsimd.dma_gather(
                            out_ap=msgs[:].rearrange("p (s e) -> p s e", e=elem),
                            in_ap=table[lo:hi, :],
                            idxs_ap=idx_sb[:, ic : ic + sub // 16],
                            num_idxs=sub,
                            num_idxs_reg=sub,
                            elem_size=elem,
                            single_packet=False,
                        )
                        for k2 in range(sub // P):
                            col = 2 * (mb + k2)
                            S = spool.tile([P, P], f32, tag="S")
                            nc.vector.tensor_scalar(
                                S[:],
                                iota_f[:],
                                meta_sb[:, col : col + 1],
                                meta_sb[:, col + 1 : col + 2],
                                eq,
                                mul,
                            )
                            nc.tensor.matmul(
                                agg_ps[:],
                                lhsT=msgs[:, k2 * elem : (k2 + 1) * elem],
                                rhs=S[:],
                                start=(bin_i == 0),
                                stop=(bin_i == nbins_b - 1),
                            )
                            bin_i += 1
                    epilogue(b, None, agg_ps)

            # ---------------- layer 1 ----------------
            def epi1(b, agg_sb, agg_ps):
                if agg_sb is None:
                    agg_sb = spool.tile([IN, P], f32, tag="aggsb")
                    nc.vector.tensor_copy(agg_sb[:], agg_ps[:])
                z_ps = psB.tile([HID, P], f32, tag="zp")
                nc.tensor.matmul(z_ps[:], lhsT=w1_sb[:], rhs=agg_sb[:],
                                 start=True, stop=True)
                h_sb = hpool.tile([HID, P], f32, tag="h")
                nc.scalar.activation(
                    h_sb[:], z_ps[:], mybir.ActivationFunctionType.Relu,
                    bias=b1_sb[:, 0:1], scale=1.0,
                )
                h_tiles.append(h_sb)

            layer(l1, xt, IN, idx1, met1, IN, epi1)

            # ------------- p = W2^T h, transpose, allgather ---------------
            for b in range(B):
                p_ps = psB.tile([OUT, P], f32, tag="zp")
                nc.tensor.matmul(p_ps[:], lhsT=w2_sb[:, :OUT], rhs=h_tiles[b][:],
                                 start=True, stop=True)
                pT = spool.tile([OUT, P], f32, tag="pT")
                nc.vector.tensor_copy(pT[:], p_ps[:])
                tr_ps = psC.tile([P, OUT], f32, tag="tr")
                nc.tensor.transpose(tr_ps[:], pT[:], idf[:OUT, :OUT])
                nc.vector.tensor_copy(rows_st[:, b * OUT : (b + 1) * OUT], tr_ps[:])
            nc.sync.dma_start(p_shard[:].rearrange("(b p) f -> p b f", p=P),
                              rows_st[:])
            nc.gpsimd.collective_compute(
                "AllGather",
                mybir.AluOpType.bypass,
                replica_groups=[list(range(NCORES))],
                ins=[p_shard[:]],
                outs=[p_full[:]],
            )

            # ---------------- layer 2 ----------------
            def epi2(b, agg_sb, agg_ps):
                o_sb = spool.tile([OUT, P], f32, tag="osb")
                src_ap = agg_ps if agg_sb is None else agg_sb
                nc.vector.tensor_scalar(o_sb[:], src_ap[:], b2_sb[:, 0:1],
                                        None, add)
                tr_ps = psC.tile([P, OUT], f32, tag="tr")
                nc.tensor.transpose(tr_ps[:], o_sb[:], idf[:OUT, :OUT])
                nc.vector.tensor_copy(rows_st[:, b * OUT : (b + 1) * OUT], tr_ps[:])

            layer(l2, p_full, OUT, idx2, met2, OUT, epi2)
            nc.sync.dma_start(outt[:].rearrange("(b p) f -> p b f", p=P),
                              rows_st[:])

    nc.compile()
    return nc


# --------------------------------------------------------------------------
# optional NTFF tracing (dev only; registers the axon profile hook)
# --------------------------------------------------------------------------

def _install_trace_shim():
    try:
        if "antenv.axon_hooks" in sys.modules:
            return True
        import antenv

        mod = types.ModuleType("antenv.axon_hooks")
        mod._hook = None
        mod.set_axon_ntff_profile_hook = lambda h: setattr(mod, "_hook", h)
        mod.get_axon_ntff_profile_hook = lambda: mod._hook
        sys.modules["antenv.axon_hooks"] = mod
        antenv.axon_hooks = mod
        from trn_agent_boot.trn_boot import _ntff_profile_via_ctypes

        mod.set_axon_ntff_profile_hook(
            _ntff_profile_via_ctypes("/opt/axon/libaxon_pjrt.so")
        )
        import concourse.bass_utils as bu

        bu.upload_artifacts = lambda tmpdir: ""
        return True
    except Exception:
        return False


LAST_EXEC_NS = None
LAST_RESULTS = None


def kernel(x, edge_index, W1, b1, W2, b2):
    global LAST_EXEC_NS, LAST_RESULTS
    from concourse.bass_utils import run_bass_kernel_spmd

    x = np.ascontiguousarray(np.asarray(x, dtype=np.float32))
    W1 = np.ascontiguousarray(np.asarray(W1, np.float32))
    b1 = np.asarray(b1, np.float32)
    W2 = np.ascontiguousarray(np.asarray(W2, np.float32))
    b2 = np.asarray(b2, np.float32)
    N, IN = x.shape
    HID = W1.shape[1]
    OUT = W2.shape[1]

    pp = _preprocess(x, edge_index)
    B, PADN, NPC = pp["B"], pp["PADN"], pp["NPC"]

    nc = _build(N, IN, HID, OUT, B, PADN, pp["l1"], pp["l2"])

    in_maps = []
    for c in range(NCORES):
        in_maps.append(
            {
                "xt": x,
                "idx1": pp["l1"]["idx16"][c],
                "idx2": pp["l2"]["idx16"][c],
                "met1": pp["l1"]["meta"][c],
                "met2": pp["l2"]["meta"][c],
                "w1": W1,
                "w2": W2,
                "b1t": b1.reshape(HID, 1).copy(),
                "b2t": b2.reshape(OUT, 1).copy(),
            }
        )

    trace = bool(int(os.environ.get("GCN_TRACE", "0")))
    if trace:
        trace = _install_trace_shim()
    res = run_bass_kernel_spmd(
        nc, in_maps, core_ids=list(range(NCORES)), trace=trace
    )
    LAST_EXEC_NS = res.exec_time_ns
    LAST_RESULTS = res

    out = np.empty((N, OUT), np.float32)
    for c in range(NCORES):
        out[c * NPC : (c + 1) * NPC] = res.results[c]["outt"][:NPC]
    return out

